# revision 36
# baseline (speedup 1.0000x reference)
"""Multi-head attention forward on 8 Trainium2 NeuronCores.

Problem: B=8, N=1024 tokens, C=1024 channels, H=16 heads, hd=64.
Returns (out [B,N,C], attn [B,H,N,N]) matching the reference
    qkv = x @ w_qkv ; attn = softmax(q k^T / sqrt(hd)) ; out = (attn v) @ w_proj + b_proj

Sharding: pure data parallel, one batch element per core, no collectives.

Per-core layout strategy (everything transpose-free on device):
  - host sends xT = x[b].T                       [C, N]   bf16
  - qT,kT = (w_qkv[:, :2C]).T-style matmul:      lhsT=w_qkv tile, rhs=xT  -> qkT [2C, N]
  - v     = x @ w_qkv[:, 2C:]:                   lhsT=xT tile, rhs=wv     -> v   [N(keys), C]
  - S^T_h [keys, tokens] = kT_h.T-matmul:        lhsT=kT_h (K=64), rhs=qT_h
  - expS = Exp(S^T * hd^-0.5)  (ACT, bf16 out; no max subtraction, scores are O(5))
  - ctx^T_h [64, N] = v_h.T @ expS  (accumulated over key tiles); a col-packed M=1
    ones-matmul accumulates rowsum[tokens] into psum partition 64 for free
  - recip = Exp(-Ln(rowsum))  (both funcs in the natural_log_exp table set)
  - bcast recip to 128 partitions with a K=1 ones matmul; attn^T = expS * bcast (DVE
    bf16 2x mode), DMA out per key tile; ctx^T normalized by the same bcast
  - out^T = w_proj-tile.T @ ctx^T + b_proj (per-partition bias), DMA out
Host transposes attnT -> attn and outT -> out (pure data movement).
"""

import numpy as np

B, N, C, H, HD = 8, 1024, 1024, 16, 64
P = 128
KT = C // P          # 8 tiles of 128 along any C/N axis
SCALE = HD ** -0.5   # 0.125
NCORES = 8


def _patch_act_tables():
    """Make Exp and Ln resolve to the single table set that contains both
    (natural_log_exp_and_others), so the kernel emits one ACT_TABLE_LOAD
    instead of thrashing between exp_and_others and the ln set per head."""
    import concourse.mybir as mybir
    import concourse.hw_specs as hw_specs
    from concourse import bacc as bacc_mod

    if getattr(hw_specs, "_attn_tables_patched", False):
        return
    AF = mybir.ActivationFunctionType
    orig = hw_specs.get_activation_tables

    def patched(arch):
        tabs = orig(arch)
        out = {}
        for name, funcs in tabs.items():
            if name == "natural_log_exp_and_others":
                out[name] = funcs
            else:
                out[name] = {f for f in funcs if f not in (AF.Exp, AF.Ln)}
        return out

    hw_specs.get_activation_tables = patched
    hw_specs._attn_tables_patched = True
    if getattr(bacc_mod, "get_activation_tables", None) is orig:
        bacc_mod.get_activation_tables = patched


def _build():
    import concourse.mybir as mybir
    import concourse.tile as tile
    from concourse import bacc

    _patch_act_tables()

    F32 = mybir.dt.float32
    BF16 = mybir.dt.bfloat16
    AF = mybir.ActivationFunctionType

    nc = bacc.Bacc("TRN2", target_bir_lowering=False, debug=False,
                   enable_asserts=True)

    xT_d = nc.dram_tensor("xT", [C, N], BF16, kind="ExternalInput")
    wqk_d = nc.dram_tensor("wqk", [C, 2 * C], BF16, kind="ExternalInput")
    wv_d = nc.dram_tensor("wv", [C, C], BF16, kind="ExternalInput")
    wp_d = nc.dram_tensor("wp", [C, C], BF16, kind="ExternalInput")
    bp_d = nc.dram_tensor("bp", [P, KT], F32, kind="ExternalInput")
    attnT_d = nc.dram_tensor("attnT", [H, N, N], BF16, kind="ExternalOutput")
    outT_d = nc.dram_tensor("outT", [C, N], F32, kind="ExternalOutput")

    with tile.TileContext(nc) as tc:
        with tc.tile_pool(name="persist", bufs=1) as pp, \
             tc.tile_pool(name="psum", bufs=1, space="PSUM") as ps:

            # ---- persistent SBUF tensors
            qk_sb = pp.tile([P, 2 * KT, N], BF16)   # outer 0..7 = qT, 8..15 = kT
            # v with a ones-column appended per head: cols 0..63 = v, 64..65 = 1.0
            # (col 64 rides along in the ctx matmul to accumulate softmax rowsums)
            v_sb = pp.tile([P, KT, H, 66], BF16)
            wp_sb = pp.tile([P, KT, C], BF16)
            bp_sb = pp.tile([P, KT], F32)
            ctxT_sb = pp.tile([P, KT, N], BF16)
            ones_row = pp.tile([1, P], BF16)

            nc.vector.memset(v_sb[:, :, :, 64:66], 1.0)
            nc.vector.memset(ones_row[:], 1.0)

            # keep the PE HAM clock-gate warm while input DMAs land: ~25us of
            # dummy matmuls on a zeroed tile (reuses the "b" psum slot, whose
            # first real use is much later)
            warm = pp.tile([P, 512], BF16)
            nc.vector.memset(warm[:], 0.0)
            wps = ps.tile([P, N], F32, tag="s", bufs=3)
            for _ in range(40):
                nc.tensor.matmul(wps[:, 0:512], warm[:, 0:P], warm[:, :],
                                 start=True, stop=True)
            nc.sync.dma_start(out=bp_sb[:], in_=bp_d.ap())

            with tc.tile_pool(name="ph1", bufs=1) as p1:
                xT_sb = p1.tile([P, KT, N], BF16)
                wqk_sb = p1.tile([P, KT, 2 * C], BF16)
                wv_sb = p1.tile([P, KT, C], BF16)
                for k in range(KT):
                    nc.sync.dma_start(out=xT_sb[:, k, :],
                                      in_=xT_d.ap()[k * P:(k + 1) * P, :])
                # chunk order matches first use: qk(0) needs mg0, qk(8) needs
                # mg2, head-0's v fillers need wv; mg1/mg3 and wp come later
                for mg in (0, 2):
                    for k in range(KT):
                        nc.sync.dma_start(
                            out=wqk_sb[:, k, mg * 512:(mg + 1) * 512],
                            in_=wqk_d.ap()[k * P:(k + 1) * P, mg * 512:(mg + 1) * 512])
                for k in range(KT):
                    nc.sync.dma_start(out=wv_sb[:, k, :],
                                      in_=wv_d.ap()[k * P:(k + 1) * P, :])
                for mg in (1, 3):
                    for k in range(KT):
                        nc.sync.dma_start(
                            out=wqk_sb[:, k, mg * 512:(mg + 1) * 512],
                            in_=wqk_d.ap()[k * P:(k + 1) * P, mg * 512:(mg + 1) * 512])
                for k in range(KT):
                    nc.sync.dma_start(out=wp_sb[:, k, :],
                                      in_=wp_d.ap()[k * P:(k + 1) * P, :])

                def qkv_tile_parts(kind, m):
                    # One qkT/v output tile as two emission halves, so a head
                    # can wrap them around its S matmuls (keeps the PE dense
                    # across the exp-latency window at head boundaries).
                    st = {}

                    def mms(krange):
                        for k in krange:
                            for t in range(2):
                                if kind == "qk":
                                    lhsT = wqk_sb[:, k, m * P:(m + 1) * P]
                                    rhs = xT_sb[:, k, t * 512:(t + 1) * 512]
                                else:
                                    lhsT = xT_sb[:, k, m * P:(m + 1) * P]
                                    rhs = wv_sb[:, k, t * 512:(t + 1) * 512]
                                nc.tensor.matmul(
                                    st["ps"][:, t * 512:(t + 1) * 512],
                                    lhsT, rhs,
                                    start=(k == 0), stop=(k == KT - 1))

                    def part_a():
                        psA = ps.tile([P, N], F32, tag="s", bufs=3,
                                      name=f"fill_{kind}_{m}")
                        st["ps"] = psA
                        mms(range(KT // 2))

                    def part_b():
                        mms(range(KT // 2, KT))
                        # psum->sbuf copy on DVE: ACT paces phase 2, DVE has
                        # the slack
                        if kind == "qk":
                            nc.vector.tensor_copy(out=qk_sb[:, m, :],
                                                  in_=st["ps"][:, :])
                        else:
                            nc.vector.tensor_copy(
                                out=v_sb[:, m, :, 0:64],
                                in_=st["ps"][:, :].rearrange(
                                    "p (h d) -> p h d", d=HD))

                    return part_a, part_b

                def qk_tile(m):
                    a, b = qkv_tile_parts("qk", m)
                    a()
                    b()

                def dummy_parts(n_mm):
                    # pure HAM insurance: warm matmuls on the zeroed tile
                    def part():
                        wp2 = ps.tile([P, N], F32, tag="s", bufs=3,
                                      name="dummy_ps")
                        for _ in range(n_mm):
                            nc.tensor.matmul(wp2[:, 0:512], warm[:, 0:P],
                                             warm[:, :], start=True, stop=True)
                    return part

                with tc.tile_pool(name="ph2", bufs=1) as p2:

                    def finish_head(h, expS, psC):
                        # evacuate psC (Ln rowsum + raw context), reciprocal,
                        # broadcast on gpsimd, then the normalize TTs + DMAs.
                        hb = (h % 2) * 64
                        lnr = p2.tile([1, N], F32, tag="lnr", bufs=1)
                        nc.scalar.activation(lnr[:, :], psC[64:65, :], AF.Ln)
                        craw = p2.tile([64, N], BF16, tag="craw", bufs=2)
                        nc.vector.tensor_copy(out=craw[:], in_=psC[0:64, :])
                        recip = p2.tile([1, N], BF16, tag="recip", bufs=2)
                        nc.scalar.activation(recip[:, :], lnr[:, :],
                                             AF.Exp, scale=-1.0)
                        bcast = p2.tile([P, N], BF16, tag="bcast", bufs=2)
                        nc.gpsimd.partition_broadcast(bcast[:], recip[:, :])
                        nc.vector.tensor_mul(
                            out=ctxT_sb[hb:hb + 64, h // 2, :],
                            in0=craw[:, :], in1=bcast[0:64, :])
                        for i in range(KT):
                            at = p2.tile([P, N], BF16, tag="attn", bufs=3)
                            nc.vector.tensor_mul(out=at[:], in0=expS[:, i, :],
                                                 in1=bcast[:])
                            nc.sync.dma_start(
                                out=attnT_d.ap()[h, i * P:(i + 1) * P, :],
                                in_=at[:])

                    def head_pair(p, fillers):
                        # Heads A=2p (array rows 0..63) and B=2p+1 (rows
                        # 64..127): their S matmuls interleave on disjoint PE
                        # row groups, so they run concurrently and their
                        # LDWEIGHTS pull ahead. The ctx streams stay per-head
                        # sequential (keeps the ones-column rowsum trick and a
                        # single psC slot). Fillers: [0] wraps the first S
                        # pair; [1..] land after ctx_A(i); the last one's
                        # second half covers the A->B seam.
                        mq, mk = p, KT + p
                        expA = p2.tile([P, KT, N], BF16, tag="expS", bufs=2,
                                       name=f"expA_{p}")
                        expB = p2.tile([P, KT, N], BF16, tag="expS", bufs=2,
                                       name=f"expB_{p}")

                        def s_mm(i):
                            psSA = ps.tile([P, N], F32, tag="s", bufs=3,
                                           name=f"psSA_{p}_{i}")
                            psSB = ps.tile([P, N], F32, tag="s", bufs=3,
                                           name=f"psSB_{p}_{i}")
                            for t in range(2):
                                nc.tensor.matmul(
                                    psSA[:, t * 512:(t + 1) * 512],
                                    qk_sb[0:64, mk, i * P:(i + 1) * P],
                                    qk_sb[0:64, mq, t * 512:(t + 1) * 512],
                                    start=True, stop=True)
                                nc.tensor.matmul(
                                    psSB[:, t * 512:(t + 1) * 512],
                                    qk_sb[64:128, mk, i * P:(i + 1) * P],
                                    qk_sb[64:128, mq, t * 512:(t + 1) * 512],
                                    start=True, stop=True)
                            nc.scalar.activation(expA[:, i, :], psSA[:, :],
                                                 AF.Exp, scale=SCALE)
                            nc.scalar.activation(expB[:, i, :], psSB[:, :],
                                                 AF.Exp, scale=SCALE)

                        def ctx_mm(h, psC, expS, i):
                            for t in range(2):
                                nc.tensor.matmul(
                                    psC[0:65, t * 512:(t + 1) * 512],
                                    v_sb[:, i, h, 0:65],
                                    expS[:, i, t * 512:(t + 1) * 512],
                                    start=(i == 0), stop=(i == KT - 1))

                        # flatten filler halves into a queue; keep the last
                        # two halves in reserve for the A->B seam
                        q = [half for pair_f in fillers for half in pair_f]

                        def pop(n, reserve=2):
                            while n > 0 and len(q) > reserve:
                                q.pop(0)()
                                n -= 1

                        pop(1)
                        s_mm(0)
                        pop(1)
                        s_mm(1)
                        psCA = ps.tile([P, N], F32, tag="c", bufs=1,
                                       name=f"psCA_{p}")
                        for i in range(KT):
                            ctx_mm(2 * p, psCA, expA, i)
                            if i + 2 < KT:
                                s_mm(i + 2)
                            pop(2)
                        finish_head(2 * p, expA, psCA)
                        pop(2, reserve=0)          # the seam fillers
                        psCB = ps.tile([P, N], F32, tag="c", bufs=1,
                                       name=f"psCB_{p}")
                        for i in range(KT):
                            ctx_mm(2 * p + 1, psCB, expB, i)
                            pop(2, reserve=0)
                        finish_head(2 * p + 1, expB, psCB)

                    # pair 0 needs qk tiles 0/8 upfront; v tiles and the
                    # remaining qk tiles are interleaved into the pairs as PE
                    # filler work (pair p prefetches qk for pair p+1)
                    qk_tile(0)
                    qk_tile(KT)
                    fillers_by_pair = [[] for _ in range(H // 2)]
                    for m in range(KT):
                        fillers_by_pair[0].append(qkv_tile_parts("v", m))
                    for m in range(1, KT):
                        fillers_by_pair[m - 1].append(
                            qkv_tile_parts("qk", m))
                        fillers_by_pair[m - 1].append(
                            qkv_tile_parts("qk", KT + m))
                    fillers_by_pair[H // 2 - 1].append(
                        (dummy_parts(8), dummy_parts(8)))
                    fillers_by_pair[H // 2 - 1].append(
                        (dummy_parts(8), dummy_parts(8)))
                    for p in range(H // 2):
                        head_pair(p, fillers_by_pair[p])
                    dummy_parts(12)()

                    # ---- projection: outT = wp-tile.T @ ctxT + bias
                    for m in range(KT):
                        psA = ps.tile([P, N], F32, tag="s", bufs=3)
                        for k in range(KT):
                            for t in range(2):
                                nc.tensor.matmul(
                                    psA[:, t * 512:(t + 1) * 512],
                                    wp_sb[:, k, m * P:(m + 1) * P],
                                    ctxT_sb[:, k, t * 512:(t + 1) * 512],
                                    start=(k == 0), stop=(k == KT - 1))
                        ot = p2.tile([P, N], F32, tag="out", bufs=2)
                        nc.vector.tensor_scalar_add(ot[:], psA[:, :],
                                                    bp_sb[:, m:m + 1])
                        nc.sync.dma_start(out=outT_d.ap()[m * P:(m + 1) * P, :],
                                          in_=ot[:])

    nc.compile()
    return nc


def _in_maps(x, w_qkv, w_proj, b_proj):
    import ml_dtypes
    bf16 = ml_dtypes.bfloat16
    x = np.asarray(x, np.float32)
    w_qkv = np.asarray(w_qkv, np.float32)
    w_proj = np.asarray(w_proj, np.float32)
    b_proj = np.asarray(b_proj, np.float32)
    wqk = np.ascontiguousarray(w_qkv[:, :2 * C]).astype(bf16)
    wv = np.ascontiguousarray(w_qkv[:, 2 * C:]).astype(bf16)
    wp = w_proj.astype(bf16)
    bp = np.ascontiguousarray(b_proj.reshape(KT, P).T)
    return [{
        "xT": np.ascontiguousarray(x[b].T).astype(bf16),
        "wqk": wqk, "wv": wv, "wp": wp, "bp": bp,
    } for b in range(B)]


def _assemble(results):
    out = np.empty((B, N, C), np.float32)
    attn = np.empty((B, H, N, N), np.float32)
    for b in range(B):
        out[b] = results[b]["outT"].T
        attn[b] = np.asarray(results[b]["attnT"], np.float32).transpose(0, 2, 1)
    return out, attn


def kernel(x, w_qkv, w_proj, b_proj):
    from concourse.bass_utils import run_bass_kernel_spmd
    nc = _build()
    in_maps = _in_maps(x, w_qkv, w_proj, b_proj)
    res = run_bass_kernel_spmd(nc, in_maps, core_ids=list(range(NCORES)))
    return _assemble(res.results)


# revision 40
# speedup vs baseline: 1.2656x; 1.2656x over previous
"""Multi-head attention forward on 8 Trainium2 NeuronCores.

Problem: B=8, N=1024 tokens, C=1024 channels, H=16 heads, hd=64.
Returns (out [B,N,C], attn [B,H,N,N]) matching the reference
    qkv = x @ w_qkv ; attn = softmax(q k^T / sqrt(hd)) ; out = (attn v) @ w_proj + b_proj

Sharding: pure data parallel, one batch element per core, no collectives.

Per-core layout strategy (everything transpose-free on device):
  - host sends xT = x[b].T                       [C, N]   bf16
  - qT,kT = (w_qkv[:, :2C]).T-style matmul:      lhsT=w_qkv tile, rhs=xT  -> qkT [2C, N]
  - v     = x @ w_qkv[:, 2C:]:                   lhsT=xT tile, rhs=wv     -> v   [N(keys), C]
  - S^T_h [keys, tokens] = kT_h.T-matmul:        lhsT=kT_h (K=64), rhs=qT_h
  - expS = Exp(S^T * hd^-0.5)  (ACT, bf16 out; no max subtraction, scores are O(5))
  - ctx^T_h [64, N] = v_h.T @ expS  (accumulated over key tiles); a col-packed M=1
    ones-matmul accumulates rowsum[tokens] into psum partition 64 for free
  - recip = Exp(-Ln(rowsum))  (both funcs in the natural_log_exp table set)
  - bcast recip to 128 partitions via gpsimd partition_broadcast (idle engine,
    no psum); attn^T = expS * bcast (DVE bf16 2x mode), DMA out per key tile;
    ctx^T normalized by the same bcast
  - out^T = w_proj-tile.T @ ctx^T + b_proj (per-partition bias), DMA out
Host transposes attnT -> attn and outT -> out (pure data movement).
"""

import numpy as np

B, N, C, H, HD = 8, 1024, 1024, 16, 64
P = 128
KT = C // P          # 8 tiles of 128 along any C/N axis
SCALE = HD ** -0.5   # 0.125
NCORES = 8


def _patch_act_tables():
    """Make Exp and Ln resolve to the single table set that contains both
    (natural_log_exp_and_others), so the kernel emits one ACT_TABLE_LOAD
    instead of thrashing between exp_and_others and the ln set per head."""
    import concourse.mybir as mybir
    import concourse.hw_specs as hw_specs
    from concourse import bacc as bacc_mod

    if getattr(hw_specs, "_attn_tables_patched", False):
        return
    AF = mybir.ActivationFunctionType
    orig = hw_specs.get_activation_tables

    def patched(arch):
        tabs = orig(arch)
        out = {}
        for name, funcs in tabs.items():
            if name == "natural_log_exp_and_others":
                out[name] = funcs
            else:
                out[name] = {f for f in funcs if f not in (AF.Exp, AF.Ln)}
        return out

    hw_specs.get_activation_tables = patched
    hw_specs._attn_tables_patched = True
    if getattr(bacc_mod, "get_activation_tables", None) is orig:
        bacc_mod.get_activation_tables = patched


def _build():
    import concourse.mybir as mybir
    import concourse.tile as tile
    from concourse import bacc

    _patch_act_tables()

    F32 = mybir.dt.float32
    BF16 = mybir.dt.bfloat16
    AF = mybir.ActivationFunctionType

    nc = bacc.Bacc("TRN2", target_bir_lowering=False, debug=False,
                   enable_asserts=True)

    xT_d = nc.dram_tensor("xT", [C, N], BF16, kind="ExternalInput")
    wqk_d = nc.dram_tensor("wqk", [C, 2 * C], BF16, kind="ExternalInput")
    wv_d = nc.dram_tensor("wv", [C, C], BF16, kind="ExternalInput")
    wp_d = nc.dram_tensor("wp", [C, C], BF16, kind="ExternalInput")
    bp_d = nc.dram_tensor("bp", [P, KT], F32, kind="ExternalInput")
    attnT_d = nc.dram_tensor("attnT", [H, N, N], BF16, kind="ExternalOutput")
    outT_d = nc.dram_tensor("outT", [C, N], F32, kind="ExternalOutput")

    with tile.TileContext(nc) as tc:
        with tc.tile_pool(name="persist", bufs=1) as pp, \
             tc.tile_pool(name="psum", bufs=1, space="PSUM") as ps:

            # ---- persistent SBUF tensors
            qk_sb = pp.tile([P, 2 * KT, N], BF16)   # outer 0..7 = qT, 8..15 = kT
            # v with a ones-column appended per head: cols 0..63 = v, 64..65 = 1.0
            # (col 64 rides along in the ctx matmul to accumulate softmax rowsums)
            v_sb = pp.tile([P, KT, H, 66], BF16)
            wp_sb = pp.tile([P, KT, C], BF16)
            bp_sb = pp.tile([P, KT], F32)
            ctxT_sb = pp.tile([P, KT, N], BF16)
            ones_row = pp.tile([1, P], BF16)

            nc.vector.memset(v_sb[:, :, :, 64:66], 1.0)
            nc.vector.memset(ones_row[:], 1.0)

            # keep the PE HAM clock-gate warm while input DMAs land: ~25us of
            # dummy matmuls on a zeroed tile (reuses the "b" psum slot, whose
            # first real use is much later)
            warm = pp.tile([P, 512], BF16)
            nc.vector.memset(warm[:], 0.0)
            wps = ps.tile([P, N], F32, tag="s", bufs=2)
            for _ in range(32):
                nc.tensor.matmul(wps[:, 0:512], warm[:, 0:P], warm[:, :],
                                 start=True, stop=True)
            nc.sync.dma_start(out=bp_sb[:], in_=bp_d.ap())

            with tc.tile_pool(name="ph1", bufs=1) as p1:
                xT_sb = p1.tile([P, KT, N], BF16)
                wqk_sb = p1.tile([P, KT, 2 * C], BF16)
                wv_sb = p1.tile([P, KT, C], BF16)
                for k in range(KT):
                    nc.sync.dma_start(out=xT_sb[:, k, :],
                                      in_=xT_d.ap()[k * P:(k + 1) * P, :])
                # chunk order matches first use: qk(0) needs mg0, qk(8) needs
                # mg2, head-0's v fillers need wv; mg1/mg3 and wp come later
                for mg in (0, 2):
                    for k in range(KT):
                        nc.sync.dma_start(
                            out=wqk_sb[:, k, mg * 512:(mg + 1) * 512],
                            in_=wqk_d.ap()[k * P:(k + 1) * P, mg * 512:(mg + 1) * 512])
                for k in range(KT):
                    nc.sync.dma_start(out=wv_sb[:, k, :],
                                      in_=wv_d.ap()[k * P:(k + 1) * P, :])
                for mg in (1, 3):
                    for k in range(KT):
                        nc.sync.dma_start(
                            out=wqk_sb[:, k, mg * 512:(mg + 1) * 512],
                            in_=wqk_d.ap()[k * P:(k + 1) * P, mg * 512:(mg + 1) * 512])
                for k in range(KT):
                    nc.sync.dma_start(out=wp_sb[:, k, :],
                                      in_=wp_d.ap()[k * P:(k + 1) * P, :])

                def qkv_tile_parts(kind, m):
                    # One qkT/v output tile as two emission halves, so a head
                    # can wrap them around its S matmuls (keeps the PE dense
                    # across the exp-latency window at head boundaries).
                    st = {}

                    def mms(krange):
                        for k in krange:
                            for t in range(2):
                                if kind == "qk":
                                    lhsT = wqk_sb[:, k, m * P:(m + 1) * P]
                                    rhs = xT_sb[:, k, t * 512:(t + 1) * 512]
                                else:
                                    lhsT = xT_sb[:, k, m * P:(m + 1) * P]
                                    rhs = wv_sb[:, k, t * 512:(t + 1) * 512]
                                nc.tensor.matmul(
                                    st["ps"][:, t * 512:(t + 1) * 512],
                                    lhsT, rhs,
                                    start=(k == 0), stop=(k == KT - 1))

                    def part_a():
                        psA = ps.tile([P, N], F32, tag="s", bufs=2,
                                      name=f"fill_{kind}_{m}")
                        st["ps"] = psA
                        mms(range(KT // 2))

                    def part_b():
                        mms(range(KT // 2, KT))
                        # psum->sbuf copy on DVE: ACT paces phase 2, DVE has
                        # the slack
                        if kind == "qk":
                            nc.vector.tensor_copy(out=qk_sb[:, m, :],
                                                  in_=st["ps"][:, :])
                        else:
                            nc.vector.tensor_copy(
                                out=v_sb[:, m, :, 0:64],
                                in_=st["ps"][:, :].rearrange(
                                    "p (h d) -> p h d", d=HD))

                    return part_a, part_b

                def qk_tile(m):
                    a, b = qkv_tile_parts("qk", m)
                    a()
                    b()

                def dummy_parts(n_mm):
                    # pure HAM insurance: warm matmuls on the zeroed tile
                    def part():
                        wp2 = ps.tile([P, N], F32, tag="s", bufs=2,
                                      name="dummy_ps")
                        for _ in range(n_mm):
                            nc.tensor.matmul(wp2[:, 0:512], warm[:, 0:P],
                                             warm[:, :], start=True, stop=True)
                    return part

                with tc.tile_pool(name="ph2", bufs=1) as p2:

                    def head(h, fillers, evac_prev, epi_prev):
                        # Software-pipelined across heads: the previous head's
                        # psC evacuation (Ln/craw/recip) is emitted AFTER this
                        # head's first two S matmuls so ACT's exp stream never
                        # pauses at the boundary, and its epilogue (bcast
                        # matmul + normalize TTs + DMAs) lands mid-stream when
                        # recip is long since ready. Fillers are dense PE work
                        # (qk/v tiles) slotted into the keytile stream.
                        hb = (h % 2) * 64       # partition base of this head's dims
                        mq = h // 2             # qT outer index
                        mk = KT + h // 2        # kT outer index
                        expS = p2.tile([P, KT, N], BF16, tag="expS", bufs=2)
                        state = {}

                        def s_mm(i):
                            psS = ps.tile([P, N], F32, tag="s", bufs=2)
                            for t in range(2):
                                nc.tensor.matmul(
                                    psS[:, t * 512:(t + 1) * 512],
                                    qk_sb[hb:hb + 64, mk, i * P:(i + 1) * P],
                                    qk_sb[hb:hb + 64, mq, t * 512:(t + 1) * 512],
                                    start=True, stop=True)
                            nc.scalar.activation(expS[:, i, :], psS[:, :],
                                                 AF.Exp, scale=SCALE)

                        def ctx_mm(i):
                            for t in range(2):
                                nc.tensor.matmul(
                                    state["psC"][0:65, t * 512:(t + 1) * 512],
                                    v_sb[:, i, h, 0:65],
                                    expS[:, i, t * 512:(t + 1) * 512],
                                    start=(i == 0), stop=(i == KT - 1))

                        # NOTE: emission order IS dependency order for Tile;
                        # head 0's v-tile fillers must precede the ctx_mm
                        # that reads them. The first filler's halves wrap the
                        # S matmuls so the PE stays dense across the
                        # exp-latency window at the head boundary.
                        fa, fb = fillers[0] if fillers else (None, None)
                        if fa is not None:
                            fa()
                        s_mm(0)
                        if fb is not None:
                            fb()
                        s_mm(1)
                        if evac_prev is not None:
                            evac_prev()
                        if epi_prev is not None:
                            epi_prev()
                        psC = ps.tile([P, N], F32, tag="c", bufs=2,
                                      name=f"psC_{h}")
                        state["psC"] = psC
                        for i in range(KT):
                            ctx_mm(i)
                            if i + 2 < KT:
                                s_mm(i + 2)
                            if 1 + i < len(fillers):
                                f2a, f2b = fillers[1 + i]
                                f2a()
                                f2b()

                        def evacuate():
                            # frees psC: Ln takes the rowsum row, a DVE copy
                            # takes the raw context; recip = exp(-ln(rowsum))
                            lnr = p2.tile([1, N], F32, tag="lnr", bufs=1)
                            nc.scalar.activation(lnr[:, :],
                                                 state["psC"][64:65, :], AF.Ln)
                            craw = p2.tile([64, N], BF16, tag="craw", bufs=2)
                            nc.vector.tensor_copy(out=craw[:],
                                                  in_=state["psC"][0:64, :])
                            recip = p2.tile([1, N], BF16, tag="recip", bufs=2)
                            nc.scalar.activation(recip[:, :], lnr[:, :],
                                                 AF.Exp, scale=-1.0)
                            state["craw"] = craw
                            state["recip"] = recip

                        def epilogue():
                            # broadcast recip across partitions on the (idle)
                            # gpsimd engine: no psum, no PE involvement
                            bcast = p2.tile([P, N], BF16, tag="bcast", bufs=2)
                            nc.gpsimd.partition_broadcast(
                                bcast[:], state["recip"][:, :])
                            # normalized context (bf16 2x mode, SBUF only)
                            nc.vector.tensor_mul(
                                out=ctxT_sb[hb:hb + 64, h // 2, :],
                                in0=state["craw"][:, :], in1=bcast[0:64, :])
                            # normalized attention rows -> DRAM (bf16)
                            for i in range(KT):
                                at = p2.tile([P, N], BF16, tag="attn", bufs=3)
                                nc.vector.tensor_mul(out=at[:],
                                                     in0=expS[:, i, :],
                                                     in1=bcast[:])
                                nc.sync.dma_start(
                                    out=attnT_d.ap()[h, i * P:(i + 1) * P, :],
                                    in_=at[:])

                        return evacuate, epilogue

                    # head 0 needs qk tiles 0/8 upfront; v tiles and the
                    # remaining qk tiles are interleaved into the heads as
                    # PE filler work (odd head h prefetches qk for head h+1)
                    qk_tile(0)
                    qk_tile(KT)
                    fillers_by_head = [[] for _ in range(H)]
                    for m in range(KT):
                        fillers_by_head[0].append(qkv_tile_parts("v", m))
                    # one qk tile per head so every head has dense PE filler:
                    # heads 2m-2 / 2m-1 prefetch the pair for head 2m
                    for m in range(1, KT):
                        fillers_by_head[2 * m - 2].append(
                            qkv_tile_parts("qk", m))
                        fillers_by_head[2 * m - 1].append(
                            qkv_tile_parts("qk", KT + m))
                    # tail heads have no real prefetch left: a few dummy warm
                    # MMs keep the clock up into the projection
                    for h in (H - 2, H - 1):
                        fillers_by_head[h].append(
                            (dummy_parts(4), dummy_parts(4)))
                    evac, epi = None, None
                    for h in range(H):
                        evac, epi = head(h, fillers_by_head[h], evac, epi)
                    evac()
                    epi()

                    # ---- projection: outT = wp-tile.T @ ctxT + bias
                    # alternate psum tags: the "c" slots are idle by now, so
                    # this gives a 4-deep rotation for the tail
                    for m in range(KT):
                        psA = ps.tile([P, N], F32,
                                      tag="s" if m % 2 == 0 else "c", bufs=2)
                        for k in range(KT):
                            for t in range(2):
                                nc.tensor.matmul(
                                    psA[:, t * 512:(t + 1) * 512],
                                    wp_sb[:, k, m * P:(m + 1) * P],
                                    ctxT_sb[:, k, t * 512:(t + 1) * 512],
                                    start=(k == 0), stop=(k == KT - 1))
                        ot = p2.tile([P, N], F32, tag="out", bufs=2)
                        nc.vector.tensor_scalar_add(ot[:], psA[:, :],
                                                    bp_sb[:, m:m + 1])
                        nc.sync.dma_start(out=outT_d.ap()[m * P:(m + 1) * P, :],
                                          in_=ot[:])

    nc.compile()
    return nc


def _in_maps(x, w_qkv, w_proj, b_proj):
    import ml_dtypes
    bf16 = ml_dtypes.bfloat16
    x = np.asarray(x, np.float32)
    w_qkv = np.asarray(w_qkv, np.float32)
    w_proj = np.asarray(w_proj, np.float32)
    b_proj = np.asarray(b_proj, np.float32)
    wqk = np.ascontiguousarray(w_qkv[:, :2 * C]).astype(bf16)
    wv = np.ascontiguousarray(w_qkv[:, 2 * C:]).astype(bf16)
    wp = w_proj.astype(bf16)
    bp = np.ascontiguousarray(b_proj.reshape(KT, P).T)
    return [{
        "xT": np.ascontiguousarray(x[b].T).astype(bf16),
        "wqk": wqk, "wv": wv, "wp": wp, "bp": bp,
    } for b in range(B)]


def _assemble(results):
    out = np.empty((B, N, C), np.float32)
    attn = np.empty((B, H, N, N), np.float32)
    for b in range(B):
        out[b] = results[b]["outT"].T
        attn[b] = np.asarray(results[b]["attnT"], np.float32).transpose(0, 2, 1)
    return out, attn


def kernel(x, w_qkv, w_proj, b_proj):
    from concourse.bass_utils import run_bass_kernel_spmd
    nc = _build()
    in_maps = _in_maps(x, w_qkv, w_proj, b_proj)
    res = run_bass_kernel_spmd(nc, in_maps, core_ids=list(range(NCORES)))
    return _assemble(res.results)
